# revision 1
# baseline (speedup 1.0000x reference)
"""Batched Viterbi decode (CRF inference) on 8 Trainium2 NeuronCores.

Data-parallel: batch 512 split as 64 sequences per core; each core runs an
independent Viterbi scan, zero communication.

Raw-Bass program (TileContext's exit drain does not compile on this
toolchain), single straight-line DVE stream (program order = dependency
order), GPSIMD only issues DMAs:

  forward (t = 0..T-1; t=0 is a masked no-op step):
    scores[b,(j,i)] = alpha[b,i] + trans[i,j]     tensor_tensor, bcast AP
    R = running max over i per j                  tensor_tensor_scan (+reset)
    M[b,j] = R[b,(j,31)]
    lt = [R < M] ; bp[b,j] = sum_i lt             = first-argmax (jnp tie-break)
    alpha = where(sm_t, M + e_t, alpha)           copy_predicated (u8 mask)
    ring[t] = bp (u8)
  masked steps get identity bp via 32 bulk copy_predicated columns.
  backtrace: tag_t = bp_{t+1}[tag_{t+1}] via scalar_tensor_tensor:
    out=(iota==tag)*bp, accum_out=sum = gather;  paths = tag*mask; int32 out.

All adds/compares are IEEE f32 on the same operands in the same order as the
jax reference -> bit-exact results incl. argmax tie-breaking.
"""

import sys

for p in ("/opt/trn_rl_repo", "/opt/pypackages"):
    if p not in sys.path:
        sys.path.insert(0, p)

from contextlib import ExitStack

import numpy as np

import concourse.bass as bass
from concourse import mybir
from concourse.bass_utils import run_bass_kernel_spmd

A = mybir.AluOpType
DT = mybir.dt
AX = mybir.AxisListType

B, T, L = 512, 2048, 32
NCORES = 8
BL = B // NCORES  # 64 sequences per core
NEG = -1.0e30


def build_program(T_=T, chunk=32):
    assert T_ % chunk == 0
    nch = T_ // chunk
    CL = chunk * L

    nc = bass.Bass()
    x = nc.declare_dram_parameter("x", [BL, T_, L], DT.float32, isOutput=False)
    slen = nc.declare_dram_parameter("slen", [BL, 1], DT.int32, isOutput=False)
    trep = nc.declare_dram_parameter("trep", [BL, L * L], DT.float32, isOutput=False)
    rstv = nc.declare_dram_parameter("rstv", [BL, L * L], DT.float32, isOutput=False)
    iotf = nc.declare_dram_parameter("iotf", [BL, L], DT.float32, isOutput=False)
    itf = nc.declare_dram_parameter("itf", [BL, T_], DT.float32, isOutput=False)
    pout = nc.declare_dram_parameter("paths", [BL, T_], DT.int32, isOutput=True)

    xflat = x[:].rearrange("p t l -> p (t l)")

    with ExitStack() as ctx:
        e = ctx.enter_context
        trep_sb = e(nc.sbuf_tensor([BL, L * L], DT.float32))
        rstv_sb = e(nc.sbuf_tensor([BL, L * L], DT.float32))
        iotf_sb = e(nc.sbuf_tensor([BL, L], DT.float32))
        itf_sb = e(nc.sbuf_tensor([BL, T_], DT.float32))
        slen_sb = e(nc.sbuf_tensor([BL, 1], DT.int32))
        slen_f = e(nc.sbuf_tensor([BL, 1], DT.float32))
        alpha = e(nc.sbuf_tensor([BL, L], DT.float32))
        m_sb = e(nc.sbuf_tensor([BL, T_], DT.float32))
        sm_sb = e(nc.sbuf_tensor([BL, T_], DT.float32))
        nsm_sb = e(nc.sbuf_tensor([BL, T_], DT.float32))
        sm_u8 = e(nc.sbuf_tensor([BL, T_], DT.uint8))
        nsm_u8 = e(nc.sbuf_tensor([BL, T_], DT.uint8))
        iot_u8 = e(nc.sbuf_tensor([BL, L], DT.uint8))
        paths = e(nc.sbuf_tensor([BL, T_], DT.float32))
        ring = e(nc.sbuf_tensor([BL, T_ * L], DT.uint8))
        outi = e(nc.sbuf_tensor([BL, T_], DT.int32))
        xt_a = e(nc.sbuf_tensor([BL, CL], DT.float32))
        xt_b = e(nc.sbuf_tensor([BL, CL], DT.float32))
        sc = e(nc.sbuf_tensor([BL, L * L], DT.float32))
        R = e(nc.sbuf_tensor([BL, L * L], DT.float32))
        ltt = e(nc.sbuf_tensor([BL, L * L], DT.float32))
        bpf = e(nc.sbuf_tensor([BL, L], DT.float32))
        cand = e(nc.sbuf_tensor([BL, L], DT.float32))
        lt32 = e(nc.sbuf_tensor([BL, L], DT.float32))
        bsl = e(nc.sbuf_tensor([BL, L], DT.float32))
        junk = e(nc.sbuf_tensor([BL, L], DT.float32))
        tbl_sem = e(nc.semaphore("tbl_sem"))
        xa_sem = e(nc.semaphore("xa_sem"))
        xb_sem = e(nc.semaphore("xb_sem"))
        out_sem = e(nc.semaphore("out_sem"))
        dve_sem = e(nc.semaphore("dve_sem"))

        xt_ab = [xt_a, xt_b]
        trep3 = trep_sb[:].rearrange("p (j i) -> p j i", i=L)
        alpha_b = alpha[:].unsqueeze(1).broadcast_to([BL, L, L])
        R3 = R[:].rearrange("p (j i) -> p j i", i=L)
        Mv = R3[:, :, L - 1 : L]
        ltt3 = ltt[:].rearrange("p (j i) -> p j i", i=L)
        sc3 = sc[:].rearrange("p (j i) -> p j i", i=L)

        with nc.Block() as block:
            marks = {}
            total = [0]

            @block.vector
            def _(v):
                n = 0

                def S(inst):
                    nonlocal n
                    inst.then_inc(dve_sem, 1)
                    n += 1
                    return inst

                def W():
                    v.wait_ge(dve_sem, n)

                # wait for the 5 table DMAs
                v.wait_ge(tbl_sem, 16 * 5)
                # masks: m = t < len; sm = m & (t>=1); nsm = !sm
                S(v.tensor_copy(slen_f[:], slen_sb[:]))
                W()
                S(v.tensor_scalar(
                    out=m_sb[:], in0=itf_sb[:], scalar1=slen_f[:],
                    scalar2=None, op0=A.is_lt,
                ))
                S(v.tensor_scalar(
                    out=sm_sb[:], in0=itf_sb[:], scalar1=1.0,
                    scalar2=None, op0=A.is_ge,
                ))
                W()
                S(v.tensor_tensor(
                    out=sm_sb[:], in0=sm_sb[:], in1=m_sb[:], op=A.mult
                ))
                W()
                S(v.tensor_scalar(
                    out=nsm_sb[:], in0=sm_sb[:], scalar1=0.0,
                    scalar2=None, op0=A.is_equal,
                ))
                S(v.tensor_copy(sm_u8[:], sm_sb[:]))
                W()
                S(v.tensor_copy(nsm_u8[:], nsm_sb[:]))
                S(v.tensor_copy(iot_u8[:], iotf_sb[:]))

                # forward scan
                v.wait_ge(xa_sem, 16)  # chunk 0 loaded
                W()
                S(v.tensor_copy(alpha[:], xt_a[:, 0:L]))
                for c in range(nch):
                    if c > 0:
                        if c % 2 == 0:
                            v.wait_ge(xa_sem, 16 * (c // 2 + 1))
                        else:
                            v.wait_ge(xb_sem, 16 * ((c - 1) // 2 + 1))
                    xt = xt_ab[c % 2]
                    for u in range(chunk):
                        t = c * chunk + u
                        W()
                        S(v.tensor_tensor(
                            out=sc3, in0=alpha_b, in1=trep3, op=A.add
                        ))
                        W()
                        S(v.tensor_tensor_scan(
                            out=R[:], data0=rstv_sb[:], data1=sc[:],
                            initial=0.0, op0=A.add, op1=A.max,
                        ))
                        W()
                        S(v.tensor_tensor(
                            out=ltt3, in0=R3,
                            in1=Mv.broadcast_to([BL, L, L]), op=A.is_lt,
                        ))
                        S(v.tensor_tensor(
                            out=cand[:].unsqueeze(2), in0=Mv,
                            in1=xt[:, u * L : (u + 1) * L].unsqueeze(2),
                            op=A.add,
                        ))
                        W()
                        S(v.tensor_reduce(
                            out=bpf[:], in_=ltt3, axis=AX.X, op=A.add
                        ))
                        S(v.copy_predicated(
                            out=alpha[:],
                            mask=sm_u8[:, t : t + 1].broadcast_to([BL, L]),
                            data=cand[:],
                        ))
                        W()
                        S(v.tensor_copy(ring[:, t * L : (t + 1) * L], bpf[:]))
                    marks[c] = n  # all reads of chunk c done by here

                # identity backpointers on masked steps
                ring3 = ring[:].rearrange("p (t l) -> p t l", l=L)
                W()
                for l_ in range(L):
                    S(v.copy_predicated(
                        out=ring3[:, :, l_],
                        mask=nsm_u8[:],
                        data=iot_u8[:, l_ : l_ + 1].broadcast_to([BL, T_]),
                    ))

                # last = first argmax of final alpha -> paths[:, T-1]
                W()
                S(v.tensor_tensor_scan(
                    out=lt32[:], data0=rstv_sb[:, 0:L], data1=alpha[:],
                    initial=0.0, op0=A.add, op1=A.max,
                ))
                W()
                S(v.tensor_tensor(
                    out=junk[:], in0=lt32[:],
                    in1=lt32[:, L - 1 : L].broadcast_to([BL, L]), op=A.is_lt,
                ))
                W()
                S(v.tensor_reduce(
                    out=paths[:, T_ - 1 : T_], in_=junk[:], axis=AX.X, op=A.add
                ))

                # backtrace: tag_t = ring slot t+1 applied to tag_{t+1}
                for t in range(T_ - 2, -1, -1):
                    s = t + 1
                    S(v.tensor_copy(bsl[:], ring[:, s * L : (s + 1) * L]))
                    W()
                    S(v.scalar_tensor_tensor(
                        out=junk[:],
                        in0=iotf_sb[:],
                        scalar=paths[:, s : s + 1],
                        in1=bsl[:],
                        op0=A.is_equal,
                        op1=A.mult,
                        accum_out=paths[:, t : t + 1],
                    ))
                    W()

                # mask (MASK_ID = 0), cast int32, signal output DMA
                S(v.tensor_tensor(
                    out=paths[:], in0=paths[:], in1=m_sb[:], op=A.mult
                ))
                W()
                S(v.tensor_copy(outi[:], paths[:]))
                total[0] = n
                v.wait_ge(out_sem, 16)

            @block.gpsimd
            def _(g):
                # 5 table DMAs, then chunk DMAs (single SWDGE queue: in-order)
                g.dma_start(trep_sb[:], trep[:]).then_inc(tbl_sem, 16)
                g.dma_start(rstv_sb[:], rstv[:]).then_inc(tbl_sem, 16)
                g.dma_start(iotf_sb[:], iotf[:]).then_inc(tbl_sem, 16)
                g.dma_start(itf_sb[:], itf[:]).then_inc(tbl_sem, 16)
                g.dma_start(slen_sb[:], slen[:]).then_inc(tbl_sem, 16)
                for c in range(nch):
                    if c >= 2:
                        # buffer c%2 is free once DVE finished chunk c-2
                        g.wait_ge(dve_sem, marks[c - 2])
                    g.dma_start(
                        xt_ab[c % 2][:], xflat[:, c * CL : (c + 1) * CL]
                    ).then_inc(xa_sem if c % 2 == 0 else xb_sem, 16)
                # final output DMA after DVE signals completion
                g.wait_ge(dve_sem, total[0])
                g.dma_start(pout[:], outi[:]).then_inc(out_sem, 16)

    return nc


def make_tables(trans_params, T_=T):
    trep = np.tile(
        np.ascontiguousarray(trans_params.T).reshape(1, L * L), (BL, 1)
    ).astype(np.float32)
    rstv = np.zeros((BL, L, L), np.float32)
    rstv[:, :, 0] = NEG
    rstv = rstv.reshape(BL, L * L)
    iotf = np.tile(np.arange(L, dtype=np.float32), (BL, 1))
    itf = np.tile(np.arange(T_, dtype=np.float32), (BL, 1))
    return trep, rstv, iotf, itf


def prepare_in_maps(np_inputs, T_=T):
    inputs = np.asarray(np_inputs["inputs"], dtype=np.float32)
    seq_lengths = np.asarray(np_inputs["seq_lengths"], dtype=np.int32)
    trans_params = np.asarray(np_inputs["trans_params"], dtype=np.float32)

    xs = inputs.reshape(NCORES, BL, T_, L)
    ls = seq_lengths.reshape(NCORES, BL, 1)
    trep, rstv, iotf, itf = make_tables(trans_params, T_=T_)
    in_maps = [
        {
            "x": xs[k],
            "slen": ls[k],
            "trep": trep,
            "rstv": rstv,
            "iotf": iotf,
            "itf": itf,
        }
        for k in range(NCORES)
    ]
    return in_maps, None


def assemble_output(results):
    paths = np.stack([results[k]["paths"] for k in range(NCORES)], axis=0)
    return paths.reshape(B, T).astype(np.int32)


def kernel(inputs, seq_lengths, trans_params):
    nc = build_program()
    in_maps, _ = prepare_in_maps(
        {
            "inputs": inputs,
            "seq_lengths": seq_lengths,
            "trans_params": trans_params,
        }
    )
    res = run_bass_kernel_spmd(nc, in_maps, list(range(NCORES)))
    return assemble_output(res.results)



# revision 66
# speedup vs baseline: 1423.9553x; 1423.9553x over previous
"""Batched Viterbi decode (CRF inference) on 8 Trainium2 NeuronCores.

Data-parallel: batch 512 split as 64 sequences per core, zero comms.

Packed layout: partition p = h*64 + b encodes (j-half h, sequence b); the
per-step passes run on [128, 512] tiles (free dim halved vs a [64, 1024]
layout; instruction cost scales with free size only, partitions are free).

Engine split per step t:
  PE : pm[t%2] = I @ trans  (start)   -- refresh scores with the constant
       pm[t%2] += selK @ alpha_pad    -- merged-alpha broadcast (accum)
       Each element gets exactly one true-f32 nonzero product per matmul,
       so pm = fl(trans + alpha) BIT-EXACTLY (validated on HW, probe7/8).
       selK also performs the cross-partition merge of the zero-padded
       alpha halves (each half needs the other half's 16 entries).
  DVE: R = block max-scan of pm  [128,512] (rstv -1e30 reset trick)
       cand = M + emit -> abz[t%2] halves, then a write-only
       copy_predicated overlay of abz[(t-1)%2] where the step is masked
       (carries alpha through padding; WAW order is FIFO, no drain)
       every 2 steps: ltt = R2 < M2 [128,1024] is_lt, then blocked
       reduce-add -> bp counts (= first-argmax, exact tie-break)
  ACT: ring_half[pair] = bpf (u8)
  Pool: DMA issue only (this toolchain rejects compute ops on Pool).

Backtrace: ring_half reassembled to [64, 2*T*16] via 4 contiguous
partition-shift SBUF->SBUF DMAs, then one scalar_tensor_tensor gather per
step (u8 in1, 3D (th,h,jl) AP keeps strides under the 16-bit ISA limit).

Hazard rule (probe4): consecutive dependent DVE ops read stale data on
this HW unless separated by an explicit drain; drains are placed at every
producer->consumer edge (3 per step, the dependency-depth floor).

All f32 adds/compares happen on the same operands in the same order as the
jax reference -> bit-exact results incl. first-argmax tie-breaking.
"""

import sys

for p in ("/opt/trn_rl_repo", "/opt/pypackages"):
    if p not in sys.path:
        sys.path.insert(0, p)

from contextlib import ExitStack

import numpy as np

import concourse.bass as bass
from concourse import mybir
from concourse.bass_utils import run_bass_kernel_spmd

A = mybir.AluOpType
DT = mybir.dt
AX = mybir.AxisListType

B, T, L = 512, 2048, 32
NCORES = 8
BL = B // NCORES  # 64 sequences per core
H = 16  # j's per half
NEG = -1.0e30


def build_program(T_=T, chunk=32, debug=False):
    assert T_ % chunk == 0
    nch = T_ // chunk
    CH = chunk * H  # x-chunk cols

    nc = bass.Bass()
    if debug:
        dbg_rh = nc.declare_dram_parameter(
            "dbg_rh", [128, T_ * H], DT.uint8, isOutput=True
        )
        dbg_rf = nc.declare_dram_parameter(
            "dbg_rf", [BL, 2 * T_ * H], DT.uint8, isOutput=True
        )
        dbg_pf = nc.declare_dram_parameter(
            "dbg_pf", [BL, T_], DT.float32, isOutput=True
        )
        dbg_am = nc.declare_dram_parameter(
            "dbg_am", [128, L], DT.float32, isOutput=True
        )
        dbg_sm = nc.declare_dram_parameter(
            "dbg_sm", [128, T_], DT.uint8, isOutput=True
        )
        dbg_ap = nc.declare_dram_parameter(
            "dbg_ap", [128, H], DT.float32, isOutput=True
        )
    x = nc.declare_dram_parameter("x", [128, T_ * H], DT.float32, isOutput=False)
    slen = nc.declare_dram_parameter("slen", [128, 1], DT.int32, isOutput=False)
    trep = nc.declare_dram_parameter("trep", [128, H * L], DT.float32, isOutput=False)
    rstv = nc.declare_dram_parameter("rstv", [128, H * L], DT.float32, isOutput=False)
    sel = nc.declare_dram_parameter("sel", [128, 128], DT.float32, isOutput=False)
    eye = nc.declare_dram_parameter("eye", [128, 128], DT.float32, isOutput=False)
    iotf = nc.declare_dram_parameter("iotf", [128, L], DT.float32, isOutput=False)
    ioth = nc.declare_dram_parameter("ioth", [128, H], DT.uint8, isOutput=False)
    itf = nc.declare_dram_parameter("itf", [128, T_], DT.float32, isOutput=False)
    pout = nc.declare_dram_parameter("paths", [BL, T_], DT.int32, isOutput=True)

    NTBL = 8  # table DMAs (all but x chunks)

    with ExitStack() as ctx:
        e = ctx.enter_context
        trep_sb = e(nc.sbuf_tensor([128, H * L], DT.float32))
        rstv_sb = e(nc.sbuf_tensor([128, H * L], DT.float32))
        sel_sb = e(nc.sbuf_tensor([128, 128], DT.float32))
        eye_sb = e(nc.sbuf_tensor([128, 128], DT.float32))
        iotf_sb = e(nc.sbuf_tensor([128, L], DT.float32))
        ioth_sb = e(nc.sbuf_tensor([128, H], DT.uint8))
        itf_sb = e(nc.sbuf_tensor([128, T_], DT.float32))
        slen_sb = e(nc.sbuf_tensor([128, 1], DT.int32))
        slen_f = e(nc.sbuf_tensor([128, 1], DT.float32))
        m_sb = e(nc.sbuf_tensor([128, T_], DT.float32))
        mtmp = e(nc.sbuf_tensor([128, T_], DT.float32))
        sm_u8 = e(nc.sbuf_tensor([128, T_], DT.uint8))
        nsm_u8 = e(nc.sbuf_tensor([128, T_], DT.uint8))
        # zero-padded alpha ping-pong (lower partitions hold cols 0:16,
        # upper cols 16:32): state after step t lands in abz[t % 2]; the
        # masked overlay copies the old buffer in, write-only (no drain)
        abz = [
            e(nc.sbuf_tensor(f"abz{i}", [128, L], DT.float32))
            for i in range(2)
        ]
        xt_a = e(nc.sbuf_tensor([128, CH], DT.float32))
        xt_b = e(nc.sbuf_tensor([128, CH], DT.float32))
        # R holds two steps (odd step in half 0, even in half 1) so
        # islt+reduce run once per step pair on [128, 1024]
        R = e(nc.sbuf_tensor([128, 2 * H * L], DT.float32))
        ltt = e(nc.sbuf_tensor([128, 2 * H * L], DT.float32))
        bpf = [
            e(nc.sbuf_tensor(f"bpf{i}", [128, 2 * H], DT.float32))
            for i in range(2)
        ]
        ring_h = e(nc.sbuf_tensor([128, T_ * H], DT.uint8))
        # half-major: [h, t, jl] so reassembly is 2 contiguous DMAs
        ring_f = e(nc.sbuf_tensor([BL, 2 * T_ * H], DT.uint8))
        paths = e(nc.sbuf_tensor([BL, T_], DT.float32))
        outi = e(nc.sbuf_tensor([BL, T_], DT.int32))
        junk = e(nc.sbuf_tensor([BL, L], DT.float32))
        lt32 = e(nc.sbuf_tensor([BL, L], DT.float32))
        # sc in PSUM: PE refreshes trans (I @ trep) then accumulates the
        # merged alpha (selK @ abz broadcast) -- fl(trans + alpha) exactly
        pm = [
            e(nc.psum_tensor(f"pm{i}", [128, H * L], DT.float32))
            for i in range(2)
        ]
        amf = e(nc.psum_tensor("amf", [128, L], DT.float32))
        if debug:
            dbg_pf_sb = e(nc.sbuf_tensor([BL, T_], DT.float32))
            dbg_am_sb = e(nc.sbuf_tensor([128, L], DT.float32))

        tbl_sem = e(nc.semaphore("tbl_sem"))
        xa_sem = e(nc.semaphore("xa_sem"))
        xb_sem = e(nc.semaphore("xb_sem"))
        s_scan = e(nc.semaphore("s_scan"))
        s_alpha = e(nc.semaphore("s_alpha"))
        s_bpf = e(nc.semaphore("s_bpf"))
        pe_sem = e(nc.semaphore("pe_sem"))
        act_sem = e(nc.semaphore("act_sem"))
        fill_sem = e(nc.semaphore("fill_sem"))
        act2_sem = e(nc.semaphore("act2_sem"))
        asm_sem = e(nc.semaphore("asm_sem"))
        out_sem = e(nc.semaphore("out_sem"))

        xt_ab = [xt_a, xt_b]
        xsem_ab = [xa_sem, xb_sem]
        R5 = R[:].rearrange("p (s j i) -> p s j i", s=2, i=L)
        ltt5 = ltt[:].rearrange("p (s j i) -> p s j i", s=2, i=L)
        ring_h3 = ring_h[:].rearrange("p (t j) -> p t j", j=H)
        npairs = (T_ - 1) // 2  # even steps close pairs (t-1, t)
        leftover = (T_ - 1) % 2 == 1  # odd final step T_-1

        with nc.Block() as block:

            @block.vector
            def _(v):
                # NOTE: consecutive dependent DVE ops read stale data on this
                # HW without an explicit drain between producer and consumer
                # (verified empirically; see probe4).
                D = v.drain

                # ---- masks ----
                v.wait_ge(tbl_sem, 16 * NTBL)
                v.tensor_copy(slen_f[:], slen_sb[:])
                D()
                v.tensor_scalar(
                    out=m_sb[:], in0=itf_sb[:], scalar1=slen_f[:],
                    scalar2=None, op0=A.is_lt,
                )
                D()
                v.tensor_scalar(
                    out=mtmp[:], in0=itf_sb[:], scalar1=1.0,
                    scalar2=None, op0=A.is_ge,
                )
                D()
                v.tensor_tensor(out=mtmp[:], in0=mtmp[:], in1=m_sb[:], op=A.mult)
                D()
                v.tensor_copy(sm_u8[:], mtmp[:])
                D()
                v.tensor_scalar(
                    out=mtmp[:], in0=mtmp[:], scalar1=0.0,
                    scalar2=None, op0=A.is_equal,
                )
                D()
                v.tensor_copy(nsm_u8[:], mtmp[:])

                # ---- alpha init from x[:, 0, :] (zero-padded halves) ----
                v.wait_ge(xa_sem, 16)
                v.memset(abz[0][:], 0.0)
                v.memset(abz[1][:], 0.0)
                D()
                v.tensor_copy(abz[0][0:64, 0:H], xt_a[0:64, 0:H])
                v.tensor_copy(
                    abz[0][64:128, H:L], xt_a[64:128, 0:H]
                ).then_inc(s_alpha, 1)

                # ---- forward scan ----
                # odd t -> R half 0, even t -> R half 1; each even step
                # closes pair (t-1, t): one islt+reduce over [128, 1024]
                for t in range(1, T_):
                    c, u = divmod(t, chunk)
                    if u == 0:
                        if c % 2 == 0:
                            v.wait_ge(xa_sem, 16 * (c // 2 + 1))
                        else:
                            v.wait_ge(xb_sem, 16 * ((c - 1) // 2 + 1))
                    xt = xt_ab[c % 2]
                    ht = (t + 1) % 2  # odd->0, even->1
                    Mv = R5[:, ht, :, L - 1 : L]
                    v.wait_ge(pe_sem, t)
                    v.tensor_tensor_scan(
                        out=R[:, ht * H * L : (ht + 1) * H * L],
                        data0=rstv_sb[:], data1=pm[t % 2][:],
                        initial=0.0, op0=A.add, op1=A.max,
                    ).then_inc(s_scan, 1)
                    D()
                    v.tensor_tensor(
                        out=abz[t % 2][0:64, 0:H].unsqueeze(2),
                        in0=Mv[0:64],
                        in1=xt[0:64, u * H : (u + 1) * H].unsqueeze(2),
                        op=A.add,
                    )
                    v.tensor_tensor(
                        out=abz[t % 2][64:128, H:L].unsqueeze(2),
                        in0=Mv[64:128],
                        in1=xt[64:128, u * H : (u + 1) * H].unsqueeze(2),
                        op=A.add,
                    )
                    v.copy_predicated(
                        out=abz[t % 2][0:64, 0:H],
                        mask=nsm_u8[0:64, t : t + 1].broadcast_to([64, H]),
                        data=abz[(t - 1) % 2][0:64, 0:H],
                    )
                    v.copy_predicated(
                        out=abz[t % 2][64:128, H:L],
                        mask=nsm_u8[64:128, t : t + 1].broadcast_to([64, H]),
                        data=abz[(t - 1) % 2][64:128, H:L],
                    ).then_inc(s_alpha, 1)
                    # bp work fills the PE merge window after pred:
                    # even t: islt2 of pair (t-1, t); odd t: reduce2 of
                    # the pair closed at t-1
                    if t % 2 == 0:
                        v.tensor_tensor(
                            out=ltt5, in0=R5,
                            in1=R5[:, :, :, L - 1 : L].broadcast_to(
                                [128, 2, H, L]
                            ),
                            op=A.is_lt,
                        )
                    if t % 2 == 1 and t >= 3:
                        pair = (t - 1) // 2
                        if pair >= 3:
                            v.wait_ge(act_sem, pair - 2)
                        v.tensor_reduce(
                            out=bpf[pair % 2][:], in_=ltt5, axis=AX.X,
                            op=A.add,
                        ).then_inc(s_bpf, 1)
                # tail: if the last step is even its pair had no following
                # odd step; otherwise the final odd step is unpaired
                if (T_ - 1) % 2 == 0:
                    v.wait_ge(act_sem, max(0, npairs - 2))
                    v.tensor_reduce(
                        out=bpf[npairs % 2][:], in_=ltt5, axis=AX.X, op=A.add
                    ).then_inc(s_bpf, 1)
                if leftover:
                    lpair = npairs + 1
                    v.tensor_tensor(
                        out=ltt5[:, 0, :, :], in0=R5[:, 0, :, :],
                        in1=R5[:, 0, :, L - 1 : L].broadcast_to([128, H, L]),
                        op=A.is_lt,
                    )
                    D()
                    v.wait_ge(act_sem, lpair - 2)
                    v.tensor_reduce(
                        out=bpf[lpair % 2][:, 0:H], in_=ltt5[:, 0, :, :],
                        axis=AX.X, op=A.add,
                    ).then_inc(s_bpf, 1)

                # ---- identity backpointers on masked steps ----
                nact = npairs + (1 if leftover else 0)
                v.wait_ge(act_sem, nact)  # all ACT ring writes done
                for jl in range(H):
                    inst = v.copy_predicated(
                        out=ring_h3[:, :, jl],
                        mask=nsm_u8[:],
                        data=ioth_sb[:, jl : jl + 1].broadcast_to([128, T_]),
                    )
                    if jl == H - 1:
                        # inc must ride the last fill so the reassembly DMA
                        # cannot read ring_h before its writes drain
                        inst.then_inc(fill_sem, 1)

                # ---- last = argmax(final alpha) ----
                v.wait_ge(pe_sem, T_)
                v.tensor_tensor_scan(
                    out=lt32[:], data0=rstv_sb[0:64, 0:L],
                    data1=amf[0:64, :],
                    initial=0.0, op0=A.add, op1=A.max,
                )
                D()
                v.tensor_tensor(
                    out=junk[:], in0=lt32[:],
                    in1=lt32[:, L - 1 : L].broadcast_to([BL, L]), op=A.is_lt,
                )
                D()
                v.tensor_reduce(
                    out=paths[:, T_ - 1 : T_], in_=junk[:], axis=AX.X, op=A.add
                )
                D()

                # ---- backtrace ----
                v.wait_ge(asm_sem, 16 * 4)
                TH = T_ // 2
                rf5 = ring_f[:].rearrange(
                    "p (th h tl j) -> p th h tl j", th=2, h=2, j=H
                )
                for t in range(T_ - 2, -1, -1):
                    s = t + 1
                    th, tl = divmod(s, TH)
                    v.scalar_tensor_tensor(
                        out=junk[:].rearrange("p (h j) -> p h j", h=2),
                        in0=iotf_sb[0:64, :].rearrange(
                            "p (h j) -> p h j", h=2
                        ),
                        scalar=paths[:, s : s + 1],
                        in1=rf5[:, th, :, tl, :],
                        op0=A.is_equal,
                        op1=A.mult,
                        accum_out=paths[:, t : t + 1],
                    )
                    D()

                # ---- mask + int32 out ----
                if debug:
                    v.tensor_copy(dbg_pf_sb[:], paths[:])
                    v.tensor_copy(dbg_am_sb[:], amf[:])
                    D()
                v.tensor_tensor(
                    out=paths[:], in0=paths[:], in1=m_sb[0:64, :], op=A.mult
                )
                D()
                v.tensor_copy(outi[:], paths[:]).then_inc(s_scan, 1)
                v.wait_ge(out_sem, 16 * (7 if debug else 1))

            @block.tensor
            def _(t_):
                # per step t: refresh trans into pm[t%2], then accumulate
                # the merged alpha (one nonzero product per element plus
                # zeros -> fl(trans + alpha) bit-exactly)
                t_.wait_ge(tbl_sem, 16 * NTBL)
                for t in range(1, T_):
                    if t >= 3:
                        # pm bank t%2 last read by scan_{t-2}
                        t_.wait_ge(s_scan, t - 2)
                    t_.matmul(
                        out=pm[t % 2][:], lhsT=eye_sb[:], rhs=trep_sb[:],
                        start=True, stop=False,
                    )
                    t_.wait_ge(s_alpha, t)
                    t_.matmul(
                        out=pm[t % 2][:],
                        lhsT=sel_sb[:],
                        rhs=abz[(t - 1) % 2][:].unsqueeze(1).broadcast_to(
                            [128, H, L]
                        ),
                        start=False, stop=True, skip_group_check=True,
                    ).then_inc(pe_sem, 1)
                # final merged alpha for the last-argmax
                t_.wait_ge(s_alpha, T_)
                t_.matmul(
                    out=amf[:], lhsT=sel_sb[:], rhs=abz[(T_ - 1) % 2][:],
                    start=True, stop=True,
                ).then_inc(pe_sem, 1)

            @block.scalar
            def _(a):
                for i in range(1, npairs + 1):
                    a.wait_ge(s_bpf, i)
                    a.copy(
                        ring_h[:, (2 * i - 1) * H : (2 * i + 1) * H],
                        bpf[i % 2][:],
                    ).then_inc(act_sem, 1)
                if leftover:
                    a.wait_ge(s_bpf, npairs + 1)
                    a.copy(
                        ring_h[:, (T_ - 1) * H : T_ * H],
                        bpf[(npairs + 1) % 2][:, 0:H],
                    ).then_inc(act_sem, 1)

            @block.gpsimd
            def _(g):
                g.dma_start(trep_sb[:], trep[:]).then_inc(tbl_sem, 16)
                g.dma_start(rstv_sb[:], rstv[:]).then_inc(tbl_sem, 16)
                g.dma_start(sel_sb[:], sel[:]).then_inc(tbl_sem, 16)
                g.dma_start(eye_sb[:], eye[:]).then_inc(tbl_sem, 16)
                g.dma_start(iotf_sb[:], iotf[:]).then_inc(tbl_sem, 16)
                g.dma_start(ioth_sb[:], ioth[:]).then_inc(tbl_sem, 16)
                g.dma_start(itf_sb[:], itf[:]).then_inc(tbl_sem, 16)
                g.dma_start(slen_sb[:], slen[:]).then_inc(tbl_sem, 16)
                for c in range(nch):
                    if c >= 2:
                        # buffer c%2 free once DVE finished chunk c-2
                        # (pred of last step of chunk c-2 -> s_alpha count)
                        g.wait_ge(s_alpha, (c - 1) * chunk)
                    g.dma_start(
                        xt_ab[c % 2][:], x[:, c * CH : (c + 1) * CH]
                    ).then_inc(xsem_ab[c % 2], 16)
                # ring reassembly after identity fill (contiguous copies)
                # ring_f layout: (th, h, tl, jl), th/h halves of 16KB each
                g.wait_ge(fill_sem, 1)
                HB = (T_ // 2) * H  # bytes per (th, h) block
                for th in range(2):
                    for h in range(2):
                        g.dma_start(
                            ring_f[:, (th * 2 + h) * HB : (th * 2 + h + 1) * HB],
                            ring_h[h * 64 : (h + 1) * 64, th * HB : (th + 1) * HB],
                        ).then_inc(asm_sem, 16)
                # final output
                g.wait_ge(s_scan, T_)
                if debug:
                    g.dma_start(dbg_rh[:], ring_h[:]).then_inc(out_sem, 16)
                    g.dma_start(dbg_rf[:], ring_f[:]).then_inc(out_sem, 16)
                    g.dma_start(dbg_pf[:], dbg_pf_sb[:]).then_inc(out_sem, 16)
                    g.dma_start(dbg_am[:], dbg_am_sb[:]).then_inc(out_sem, 16)
                    g.dma_start(dbg_sm[:], sm_u8[:]).then_inc(out_sem, 16)
                    g.dma_start(
                        dbg_ap[:, 0:H], abz[(T_ - 1) % 2][0:64, 0:H]
                    ).then_inc(out_sem, 16)
                g.dma_start(pout[:], outi[:]).then_inc(out_sem, 16)

    return nc


def make_tables(trans_params, T_=T):
    tt = np.ascontiguousarray(np.asarray(trans_params, np.float32).T)  # [j, i]
    tt = tt.reshape(2, H, L)  # [h, jl, i]
    trep = np.empty((128, H * L), np.float32)
    trep[0:64] = tt[0].reshape(1, H * L)
    trep[64:128] = tt[1].reshape(1, H * L)

    rstv = np.zeros((128, H, L), np.float32)
    rstv[:, :, 0] = NEG
    rstv = rstv.reshape(128, H * L)

    # selK sums both partition halves of the zero-padded alpha: exactly
    # one nonzero product per output element (the other half is 0)
    sel = np.zeros((128, 128), np.float32)
    for m in range(128):
        sel[m % 64, m] = 1.0
        sel[64 + m % 64, m] = 1.0
    eye = np.eye(128, dtype=np.float32)

    iotf = np.tile(np.arange(L, dtype=np.float32), (128, 1))
    ioth = np.empty((128, H), np.uint8)
    ioth[0:64] = np.arange(H, dtype=np.uint8).reshape(1, H)
    ioth[64:128] = (H + np.arange(H, dtype=np.uint8)).reshape(1, H)
    itf = np.tile(np.arange(T_, dtype=np.float32), (128, 1))
    return trep, rstv, sel, eye, iotf, ioth, itf


def prepare_in_maps(np_inputs, T_=T):
    inputs = np.asarray(np_inputs["inputs"], dtype=np.float32)
    seq_lengths = np.asarray(np_inputs["seq_lengths"], dtype=np.int32)
    trans_params = np.asarray(np_inputs["trans_params"], dtype=np.float32)

    # x_half[k][h*64+b, t*16+jl] = inputs[64k+b, t, 16h+jl]
    arr = inputs.reshape(NCORES, BL, T_, 2, H)
    ls = seq_lengths.reshape(NCORES, BL, 1)
    trep, rstv, sel, eye, iotf, ioth, itf = make_tables(trans_params, T_=T_)
    in_maps = []
    for k in range(NCORES):
        xh = np.ascontiguousarray(
            arr[k].transpose(2, 0, 1, 3)
        ).reshape(128, T_ * H)
        sl = np.concatenate([ls[k], ls[k]], axis=0)  # replicate both halves
        in_maps.append(
            {
                "x": xh,
                "slen": sl,
                "trep": trep,
                "rstv": rstv,
                "sel": sel,
                "eye": eye,
                "iotf": iotf,
                "ioth": ioth,
                "itf": itf,
            }
        )
    return in_maps, None


def assemble_output(results):
    paths = np.stack([results[k]["paths"] for k in range(NCORES)], axis=0)
    return paths.reshape(B, T).astype(np.int32)


def kernel(inputs, seq_lengths, trans_params):
    nc = build_program()
    in_maps, _ = prepare_in_maps(
        {
            "inputs": inputs,
            "seq_lengths": seq_lengths,
            "trans_params": trans_params,
        }
    )
    res = run_bass_kernel_spmd(nc, in_maps, list(range(NCORES)))
    return assemble_output(res.results)
